# revision 7
# baseline (speedup 1.0000x reference)
"""CascadeCornerPooling TRN2 kernel.

Data-parallel over batch: 16 images across 8 NeuronCores (2 per core).
Per image (NCHW, C_in=256, C_out=128, H=W=128):
    up    = relu(bn1(conv3x3(x, w_up)))
    up    = reverse-cummax over H          (TopPool)
    down  = relu(bn2(conv3x3(x, w_down)))
    merge = bn3(conv3x3(up + down, w_p))
    out   = reverse-cummax over W          (LeftPool)

Implementation: H-bands of 16 rows processed bottom-up.
 - convs: 9 shifted matmuls per Cin-chunk in float32r (full PE rate),
   accumulated in PSUM; rhs APs read padded SBUF tiles (row stride 132,
   interior at col offset 2, zeroed pad cols 1/130).
 - BN+ReLU fused into ScalarE PSUM evacuation (per-channel scale/bias APs).
 - TopPool: in-place log-shift suffix-max over 17 rows (16 band rows +
   carry row holding the pooled first row of the band below).
 - LeftPool: BN3 evacuation writes each row W-reversed with +M offset;
   one masked tensor_tensor_scan (state = max(mask*state, data)) per band
   does the segmented reverse cummax; ScalarE un-reverses and subtracts M.
"""

import numpy as np

import concourse.bass as bass
import concourse.tile as tile
from concourse import mybir
from concourse.bass_utils import run_bass_kernel_spmd

F32 = mybir.dt.float32
F32R = mybir.dt.float32r

N_CORES = 8
IMG_PER_CORE = 2
CIN, COUT = 256, 128
H = W = 128
P = 128          # partitions
R = 16           # band rows
NB = H // R      # bands per image
WP = W + 4       # padded row stride (input col w at offset 2+w; cols 1,130 zero)
HP = R + 2       # band rows + 2 halo rows
M_OFF = 128.0    # positivity offset for the LeftPool scan
EPS = 1e-5


def _split_waits(nc, max_waits=1):
    """This container's walrus rejects >1 sync-wait per instruction; hoist
    excess waits onto same-engine NOPs inserted just before."""
    for f in nc.m.functions:
        for b in f.blocks:
            new_insts = []
            for inst in b.instructions:
                si = inst.sync_info
                if si is not None and si.on_wait and len(si.on_wait) > max_waits:
                    waits = list(si.on_wait)
                    head, tail_w = waits[:-max_waits], waits[-max_waits:]
                    for ci in range(0, len(head), max_waits):
                        new_insts.append(
                            mybir.InstNoOp(
                                name=f"{inst.name}-wsplit{ci}",
                                engine=inst.engine,
                                bass_nofuse=True,
                                sync_info=mybir.SyncInfo(
                                    on_wait=head[ci : ci + max_waits], on_update=[]
                                ),
                            )
                        )
                    inst.sync_info = mybir.SyncInfo(
                        on_wait=tail_w, on_update=list(si.on_update)
                    )
                new_insts.append(inst)
            b.instructions[:] = new_insts


def build_nc(nrep=1):
    nc = bass.Bass("TRN2", target_bir_lowering=False, debug=False)

    x_d = nc.dram_tensor("x", [IMG_PER_CORE, CIN, H, W], F32R, kind="ExternalInput").ap()
    wu_d = nc.dram_tensor("wu", [P, 2 * 9 * P], F32R, kind="ExternalInput").ap()
    wd_d = nc.dram_tensor("wd", [P, 2 * 9 * P], F32R, kind="ExternalInput").ap()
    wp_d = nc.dram_tensor("wp", [P, 9 * P], F32R, kind="ExternalInput").ap()
    bn_d = nc.dram_tensor("bn", [P, 6], F32, kind="ExternalInput").ap()  # s1 b1 s2 b2 s3 b3m
    y_d = nc.dram_tensor("y", [IMG_PER_CORE, COUT, H, W], F32, kind="ExternalOutput").ap()

    with tile.TileContext(nc) as tc:
        with (
            tc.tile_pool(name="const", bufs=1) as cp,
            tc.tile_pool(name="band", bufs=1) as bp,
            tc.tile_pool(name="ps", bufs=2, space="PSUM") as ps,
        ):
            # ---- constants (wu first: first conv needs it immediately) ----
            wu_t = cp.tile([P, 2 * 9 * P], F32R)
            nc.sync.dma_start(wu_t[:], wu_d[:])
            bn_early = True
            wd_t = cp.tile([P, 2 * 9 * P], F32R)
            wp_t = cp.tile([P, 9 * P], F32R)
            wu_v = wu_t.rearrange("k (c s m) -> k c s m", c=2, s=9, m=P)
            wd_v = wd_t.rearrange("k (c s m) -> k c s m", c=2, s=9, m=P)
            wp_v = wp_t.rearrange("k (s m) -> k s m", s=9, m=P)

            bn_t = cp.tile([P, 6], F32)
            nc.sync.dma_start(bn_t[:], bn_d[:])
            nc.sync.dma_start(wd_t[:], wd_d[:])
            nc.sync.dma_start(wp_t[:], wp_d[:])
            s1, b1 = bn_t[:, 0:1], bn_t[:, 1:2]
            s2, b2 = bn_t[:, 2:3], bn_t[:, 3:4]
            s3, b3m = bn_t[:, 4:5], bn_t[:, 5:6]

            zeros = cp.tile([P, 2 * WP], F32)
            nc.vector.memset(zeros[:], 0.0)
            zv = zeros.rearrange("p (r c) -> p r c", r=2, c=WP)

            mask = cp.tile([P, R * W], F32)
            nc.vector.memset(mask[:], 1.0)
            nc.vector.memset(mask[:, 0::W], 0.0)

            # ---- band tiles (manual ping-pong) ----
            xp = [[bp.tile([P, HP * WP], F32R, name=f"xp{c}{j}", tag=f"xp{c}{j}") for j in range(2)] for c in range(2)]
            ub = [bp.tile([P, 17 * W], F32, name=f"ub{j}", tag=f"ub{j}") for j in range(2)]
            dn = [bp.tile([P, R * W], F32, name=f"dn{j}", tag=f"dn{j}") for j in range(2)]
            mg = [bp.tile([P, HP * WP], F32R, name=f"mg{j}", tag=f"mg{j}") for j in range(2)]
            sc = [bp.tile([P, R * W], F32, name=f"sc{j}", tag=f"sc{j}") for j in range(2)]
            ob = [bp.tile([P, R * W], F32, name=f"ob{j}", tag=f"ob{j}") for j in range(2)]
            sc0 = bp.tile([P, W], F32)
            ob0 = bp.tile([P, W], F32)

            # zero the pad columns (1 and WP-2) of padded tiles once;
            # DMAs/adds only ever write interiors afterwards.
            for t_ in [xp[0][0], xp[0][1], xp[1][0], xp[1][1], mg[0], mg[1]]:
                v = t_.rearrange("p (r c) -> p r c", r=HP, c=WP)
                nc.vector.tensor_copy(
                    v[:, :, 1 : WP - 1 : WP - 3],
                    zeros.rearrange("p (r c) -> p r c", r=WP, c=2)[:, 0:HP, :],
                )

            def xp_view(c, j):
                return xp[c][j].rearrange("p (r c) -> p r c", r=HP, c=WP)

            def mg_view(j):
                return mg[j].rearrange("p (r c) -> p r c", r=HP, c=WP)

            rep_ctx = tc.For_i(0, nrep, 1) if nrep > 1 else None

            def conv_mms(psum, w_chunks, src_views, t0, nrows, first_extra=None):
                """Accumulate 9*len(chunks) matmuls into psum.
                src row slot for out-row r (band-relative) and kh is r+kh."""
                n_ch = len(w_chunks)
                for ci in range(n_ch):
                    for kh in range(3):
                        for kw in range(3):
                            s = kh * 3 + kw
                            rhs = src_views[ci][:, t0 + kh : t0 + kh + nrows, 1 + kw : 1 + kw + W]
                            nc.tensor.matmul(
                                psum[:, : nrows * W],
                                w_chunks[ci][:, s, :],
                                rhs,
                                start=(ci == 0 and s == 0),
                                stop=(ci == n_ch - 1 and s == 8),
                            )

            if rep_ctx is not None:
                rep_ctx.__enter__()

            def conv3_band(n, k):
                """conv3 + LeftPool + output DMA for band k (lagged one band)."""
                h0 = H - (k + 1) * R
                j = k % 2
                mv = mg_view(j)
                n_out = R if k > 0 else R - 1
                q = 0
                while q < n_out:
                    nr = min(4, n_out - q)
                    pc = ps.tile([P, 4 * W], F32, name="pc", tag="pc")
                    conv_mms(pc, [wp_v], [mv], q, nr)
                    scv = sc[j].rearrange("p (r c) -> p r c", r=R, c=W)
                    nc.scalar.activation(
                        scv[:, q : q + nr, ::-1], pc[:, : nr * W],
                        mybir.ActivationFunctionType.Identity, bias=b3m, scale=s3,
                    )
                    q += nr
                ne = n_out * W
                nc.vector.tensor_tensor_scan(
                    sc[j][:, :ne], mask[:, :ne], sc[j][:, :ne], 0.0,
                    op0=mybir.AluOpType.mult, op1=mybir.AluOpType.max,
                )
                obv = ob[j].rearrange("p (r c) -> p r c", r=R, c=W)
                nc.scalar.activation(
                    obv[:, 0:n_out, ::-1],
                    sc[j].rearrange("p (r c) -> p r c", r=R, c=W)[:, 0:n_out, :],
                    mybir.ActivationFunctionType.Copy, bias=-M_OFF, scale=1.0,
                )
                nc.sync.dma_start(
                    y_d[n, :, h0 + 1 : h0 + 1 + n_out, :], obv[:, 0:n_out, :]
                )

            for n in range(IMG_PER_CORE):
                for k in range(NB):
                    h0 = H - (k + 1) * R
                    j = k % 2

                    # ---- DMA x band (rows h0-1 .. h0+R, clipped) ----
                    lo = max(h0 - 1, 0)
                    hi = min(h0 + R, H - 1)
                    slo = lo - (h0 - 1)          # first valid slot
                    nrows_in = hi - lo + 1
                    for c in range(2):
                        xv = xp_view(c, j)
                        nc.sync.dma_start(
                            xv[:, slo : slo + nrows_in, 2 : 2 + W],
                            x_d[n, c * P : (c + 1) * P, lo : hi + 1, :],
                        )
                        if k == 0:      # bottom band: slot HP-1 is h=H -> zero
                            nc.vector.tensor_copy(xv[:, HP - 1, 1 : WP - 1], zeros[:, 0 : WP - 2])
                        if k == NB - 1:  # top band: slot 0 is h=-1 -> zero
                            nc.vector.tensor_copy(xv[:, 0, 1 : WP - 1], zeros[:, 0 : WP - 2])

                    xviews = [xp_view(0, j), xp_view(1, j)]

                    # ---- conv1 -> ub rows 0..15 (fp32, BN+ReLU) ----
                    for t in range(R // 4):
                        pu = ps.tile([P, 4 * W], F32, name="pu", tag="pu")
                        conv_mms(pu, [wu_v[:, 0], wu_v[:, 1]], xviews, 4 * t, 4)
                        nc.scalar.activation(
                            ub[j][:, 4 * t * W : 4 * (t + 1) * W], pu[:],
                            mybir.ActivationFunctionType.Relu, bias=b1, scale=s1,
                        )

                    # ---- carry row (slot 16) ----
                    if k == 0:
                        nc.vector.memset(ub[j][:, 16 * W :], 0.0)
                    else:
                        nc.vector.tensor_copy(ub[j][:, 16 * W :], ub[1 - j][:, 0:W])

                    # ---- TopPool: in-place suffix max over 17 rows ----
                    for s in (1, 2, 4, 8, 16):
                        nrows = 17 - s
                        nc.vector.tensor_max(
                            ub[j][:, : nrows * W],
                            ub[j][:, : nrows * W],
                            ub[j][:, s * W : 17 * W],
                        )

                    # ---- conv2 -> dn (fp32, BN+ReLU) ----
                    for t in range(R // 4):
                        pd = ps.tile([P, 4 * W], F32, name="pd", tag="pd")
                        conv_mms(pd, [wd_v[:, 0], wd_v[:, 1]], xviews, 4 * t, 4)
                        nc.scalar.activation(
                            dn[j][:, 4 * t * W : 4 * (t + 1) * W], pd[:],
                            mybir.ActivationFunctionType.Relu, bias=b2, scale=s2,
                        )

                    # ---- merge1 = pooled + down -> mg interior (f32r) ----
                    mv = mg_view(j)
                    nc.vector.tensor_add(
                        mv[:, 0:R, 2 : 2 + W], ub[j].rearrange("p (r c) -> p r c", r=17, c=W)[:, 0:R, :],
                        dn[j].rearrange("p (r c) -> p r c", r=R, c=W),
                    )
                    # halo rows 16,17 = rows 0,1 of previous band (or zeros)
                    if k == 0:
                        nc.vector.tensor_copy(mv[:, R : R + 2, 2 : 2 + W], zv[:, :, 0:W])
                    else:
                        nc.vector.tensor_copy(mv[:, R : R + 2, 2 : 2 + W], mg_view(1 - j)[:, 0:2, 2 : 2 + W])

                    # ---- conv3 for the PREVIOUS band (one-band lag gives the
                    # merge-add a full band of slack before PE needs it) ----
                    if k > 0:
                        conv3_band(n, k - 1)

                # last band's conv3 after the loop
                conv3_band(n, NB - 1)

                # ---- final pass: out row 0 (kh=0 reads h=-1: all-zero, skipped) ----
                mv = mg_view((NB - 1) % 2)
                p0 = ps.tile([P, 4 * W], F32, name="p0", tag="pc")
                for kh in (1, 2):
                    for kw in range(3):
                        nc.tensor.matmul(
                            p0[:, :W], wp_v[:, kh * 3 + kw, :],
                            mv[:, kh - 1 : kh, 1 + kw : 1 + kw + W],
                            start=(kh == 1 and kw == 0), stop=(kh == 2 and kw == 2),
                        )
                nc.scalar.activation(
                    sc0[:, ::-1], p0[:, :W],
                    mybir.ActivationFunctionType.Identity, bias=b3m, scale=s3,
                )
                nc.vector.tensor_tensor_scan(
                    sc0[:], mask[:, :W], sc0[:], 0.0,
                    op0=mybir.AluOpType.mult, op1=mybir.AluOpType.max,
                )
                nc.scalar.activation(
                    ob0[:, ::-1], sc0[:],
                    mybir.ActivationFunctionType.Copy, bias=-M_OFF, scale=1.0,
                )
                nc.sync.dma_start(y_d[n, :, 0:1, :], ob0[:].rearrange("p (r c) -> p r c", r=1, c=W))
            if rep_ctx is not None:
                rep_ctx.__exit__(None, None, None)

    _split_waits(nc, max_waits=1)
    return nc


_CACHE = {}


def _get_nc():
    if "nc" not in _CACHE:
        _CACHE["nc"] = build_nc()
    return _CACHE["nc"]


def _host_prep(w_up, up_gamma, up_beta, up_mean, up_var,
               w_down, down_gamma, down_beta, down_mean, down_var,
               w_p, p_gamma, p_beta, p_mean, p_var):
    def fold(gamma, beta, mean, var):
        inv = gamma / np.sqrt(var + EPS)
        return inv.astype(np.float32), (beta - mean * inv).astype(np.float32)

    s1, b1 = fold(up_gamma, up_beta, up_mean, up_var)
    s2, b2 = fold(down_gamma, down_beta, down_mean, down_var)
    s3, b3 = fold(p_gamma, p_beta, p_mean, p_var)
    bn = np.stack([s1, b1, s2, b2, s3, b3 + M_OFF], axis=1).astype(np.float32)

    def prep_w2(w):  # (COUT, CIN, 3, 3) -> [cin128, (chunk, s, cout128)]
        a = w.transpose(1, 2, 3, 0).reshape(2, P, 3, 3, COUT)   # (chunk,k,kh,kw,m)
        a = a.transpose(1, 0, 2, 3, 4)                          # (k,chunk,kh,kw,m)
        return np.ascontiguousarray(a.reshape(P, 2 * 9 * COUT)).astype(np.float32)

    def prep_w1(w):  # (COUT, COUT, 3, 3) -> [cin128, (s, cout128)]
        a = w.transpose(1, 2, 3, 0)                             # (k,kh,kw,m)
        return np.ascontiguousarray(a.reshape(P, 9 * COUT)).astype(np.float32)

    return prep_w2(w_up), prep_w2(w_down), prep_w1(w_p), bn


def kernel(x, w_up, up_gamma, up_beta, up_mean, up_var,
           w_down, down_gamma, down_beta, down_mean, down_var,
           w_p, p_gamma, p_beta, p_mean, p_var):
    x = np.asarray(x, dtype=np.float32)
    args = [np.asarray(a, dtype=np.float32) for a in (
        w_up, up_gamma, up_beta, up_mean, up_var,
        w_down, down_gamma, down_beta, down_mean, down_var,
        w_p, p_gamma, p_beta, p_mean, p_var)]
    wu, wd, wp, bn = _host_prep(*args)

    nc = _get_nc()
    in_maps = []
    for c in range(N_CORES):
        in_maps.append({
            "x": np.ascontiguousarray(x[c * IMG_PER_CORE : (c + 1) * IMG_PER_CORE]),
            "wu": wu, "wd": wd, "wp": wp, "bn": bn,
        })
    res = run_bass_kernel_spmd(nc, in_maps, core_ids=list(range(N_CORES)), trace=False)
    return np.concatenate([res.results[c]["y"] for c in range(N_CORES)], axis=0)


if __name__ == "__main__":
    nc = build_nc()
    n_inst = sum(len(b.instructions) for f in nc.m.functions for b in f.blocks)
    print(f"built: {n_inst} instructions")


# revision 10
# speedup vs baseline: 1.0025x; 1.0025x over previous
"""CascadeCornerPooling TRN2 kernel.

Data-parallel over batch: 16 images across 8 NeuronCores (2 per core).
Per image (NCHW, C_in=256, C_out=128, H=W=128):
    up    = relu(bn1(conv3x3(x, w_up)))
    up    = reverse-cummax over H          (TopPool)
    down  = relu(bn2(conv3x3(x, w_down)))
    merge = bn3(conv3x3(up + down, w_p))
    out   = reverse-cummax over W          (LeftPool)

Implementation: H-bands of 16 rows processed bottom-up.
 - convs: 9 shifted matmuls per Cin-chunk in float32r (full PE rate),
   accumulated in PSUM; rhs APs read padded SBUF tiles (row stride 132,
   interior at col offset 2, zeroed pad cols 1/130).
 - BN+ReLU fused into ScalarE PSUM evacuation (per-channel scale/bias APs).
 - TopPool: in-place log-shift suffix-max over 17 rows (16 band rows +
   carry row holding the pooled first row of the band below).
 - LeftPool: BN3 evacuation writes each row W-reversed with +M offset;
   one masked tensor_tensor_scan (state = max(mask*state, data)) per band
   does the segmented reverse cummax; ScalarE un-reverses and subtracts M.
"""

import numpy as np

import concourse.bass as bass
import concourse.tile as tile
from concourse import mybir
from concourse.bass_utils import run_bass_kernel_spmd

F32 = mybir.dt.float32
F32R = mybir.dt.float32r

N_CORES = 8
IMG_PER_CORE = 2
CIN, COUT = 256, 128
H = W = 128
P = 128          # partitions
R = 16           # band rows
NB = H // R      # bands per image
WP = W + 4       # padded row stride (input col w at offset 2+w; cols 1,130 zero)
HP = R + 2       # band rows + 2 halo rows
M_OFF = 128.0    # positivity offset for the LeftPool scan
EPS = 1e-5


def _split_waits(nc, max_waits=1):
    """This container's walrus rejects >1 sync-wait per instruction; hoist
    excess waits onto same-engine NOPs inserted just before."""
    for f in nc.m.functions:
        for b in f.blocks:
            new_insts = []
            for inst in b.instructions:
                si = inst.sync_info
                if si is not None and si.on_wait and len(si.on_wait) > max_waits:
                    waits = list(si.on_wait)
                    head, tail_w = waits[:-max_waits], waits[-max_waits:]
                    for ci in range(0, len(head), max_waits):
                        new_insts.append(
                            mybir.InstNoOp(
                                name=f"{inst.name}-wsplit{ci}",
                                engine=inst.engine,
                                bass_nofuse=True,
                                sync_info=mybir.SyncInfo(
                                    on_wait=head[ci : ci + max_waits], on_update=[]
                                ),
                            )
                        )
                    inst.sync_info = mybir.SyncInfo(
                        on_wait=tail_w, on_update=list(si.on_update)
                    )
                new_insts.append(inst)
            b.instructions[:] = new_insts


def build_nc(nrep=1, no_pool=False, no_scan=False, pool_engine="vector"):
    nc = bass.Bass("TRN2", target_bir_lowering=False, debug=False)

    x_d = nc.dram_tensor("x", [IMG_PER_CORE, CIN, H, W], F32R, kind="ExternalInput").ap()
    wu_d = nc.dram_tensor("wu", [P, 2 * 9 * P], F32R, kind="ExternalInput").ap()
    wd_d = nc.dram_tensor("wd", [P, 2 * 9 * P], F32R, kind="ExternalInput").ap()
    wp_d = nc.dram_tensor("wp", [P, 9 * P], F32R, kind="ExternalInput").ap()
    bn_d = nc.dram_tensor("bn", [P, 6], F32, kind="ExternalInput").ap()  # s1 b1 s2 b2 s3 b3m
    y_d = nc.dram_tensor("y", [IMG_PER_CORE, COUT, H, W], F32, kind="ExternalOutput").ap()

    with tile.TileContext(nc) as tc:
        with (
            tc.tile_pool(name="const", bufs=1) as cp,
            tc.tile_pool(name="band", bufs=1) as bp,
            tc.tile_pool(name="ps", bufs=3, space="PSUM") as ps,
        ):
            # ---- constants (wu first: first conv needs it immediately) ----
            wu_t = cp.tile([P, 2 * 9 * P], F32R)
            nc.sync.dma_start(wu_t[:], wu_d[:])
            bn_early = True
            wd_t = cp.tile([P, 2 * 9 * P], F32R)
            wp_t = cp.tile([P, 9 * P], F32R)
            wu_v = wu_t.rearrange("k (c s m) -> k c s m", c=2, s=9, m=P)
            wd_v = wd_t.rearrange("k (c s m) -> k c s m", c=2, s=9, m=P)
            wp_v = wp_t.rearrange("k (s m) -> k s m", s=9, m=P)

            bn_t = cp.tile([P, 6], F32)
            nc.sync.dma_start(bn_t[:], bn_d[:])
            nc.sync.dma_start(wd_t[:], wd_d[:])
            nc.sync.dma_start(wp_t[:], wp_d[:])
            s1, b1 = bn_t[:, 0:1], bn_t[:, 1:2]
            s2, b2 = bn_t[:, 2:3], bn_t[:, 3:4]
            s3, b3m = bn_t[:, 4:5], bn_t[:, 5:6]

            zeros = cp.tile([P, 2 * WP], F32)
            nc.vector.memset(zeros[:], 0.0)
            zv = zeros.rearrange("p (r c) -> p r c", r=2, c=WP)

            mask = cp.tile([P, R * W], F32)
            nc.vector.memset(mask[:], 1.0)
            nc.vector.memset(mask[:, 0::W], 0.0)

            # ---- band tiles (manual ping-pong) ----
            xp = [[bp.tile([P, HP * WP], F32R, name=f"xp{c}{j}", tag=f"xp{c}{j}") for j in range(2)] for c in range(2)]
            ub = [bp.tile([P, 17 * W], F32, name=f"ub{j}", tag=f"ub{j}") for j in range(2)]
            dn = [bp.tile([P, R * W], F32, name=f"dn{j}", tag=f"dn{j}") for j in range(2)]
            mg = [bp.tile([P, HP * WP], F32R, name=f"mg{j}", tag=f"mg{j}") for j in range(2)]
            sc = [bp.tile([P, R * W], F32, name=f"sc{j}", tag=f"sc{j}") for j in range(2)]
            ob = [bp.tile([P, R * W], F32, name=f"ob{j}", tag=f"ob{j}") for j in range(2)]
            sc0 = bp.tile([P, W], F32)
            ob0 = bp.tile([P, W], F32)

            # zero the pad columns (1 and WP-2) of padded tiles once;
            # DMAs/adds only ever write interiors afterwards.
            for t_ in [xp[0][0], xp[0][1], xp[1][0], xp[1][1], mg[0], mg[1]]:
                v = t_.rearrange("p (r c) -> p r c", r=HP, c=WP)
                nc.vector.tensor_copy(
                    v[:, :, 1 : WP - 1 : WP - 3],
                    zeros.rearrange("p (r c) -> p r c", r=WP, c=2)[:, 0:HP, :],
                )

            def xp_view(c, j):
                return xp[c][j].rearrange("p (r c) -> p r c", r=HP, c=WP)

            def mg_view(j):
                return mg[j].rearrange("p (r c) -> p r c", r=HP, c=WP)

            rep_ctx = tc.For_i(0, nrep, 1) if nrep > 1 else None

            def conv_mms(psum, w_chunks, src_views, t0, nrows, first_extra=None):
                """Accumulate 9*len(chunks) matmuls into psum.
                src row slot for out-row r (band-relative) and kh is r+kh."""
                n_ch = len(w_chunks)
                for ci in range(n_ch):
                    for kh in range(3):
                        for kw in range(3):
                            s = kh * 3 + kw
                            rhs = src_views[ci][:, t0 + kh : t0 + kh + nrows, 1 + kw : 1 + kw + W]
                            nc.tensor.matmul(
                                psum[:, : nrows * W],
                                w_chunks[ci][:, s, :],
                                rhs,
                                start=(ci == 0 and s == 0),
                                stop=(ci == n_ch - 1 and s == 8),
                            )

            if rep_ctx is not None:
                rep_ctx.__enter__()

            def conv3_band(n, k):
                """conv3 + LeftPool + output DMA for band k (lagged one band)."""
                h0 = H - (k + 1) * R
                j = k % 2
                mv = mg_view(j)
                n_out = R if k > 0 else R - 1
                q = 0
                while q < n_out:
                    nr = min(4, n_out - q)
                    pc = ps.tile([P, 4 * W], F32, name="pc", tag="pc", bufs=2)
                    conv_mms(pc, [wp_v], [mv], q, nr)
                    scv = sc[j].rearrange("p (r c) -> p r c", r=R, c=W)
                    nc.scalar.activation(
                        scv[:, q : q + nr, ::-1], pc[:, : nr * W],
                        mybir.ActivationFunctionType.Identity, bias=b3m, scale=s3,
                    )
                    q += nr
                ne = n_out * W
                if not no_scan:
                    nc.vector.tensor_tensor_scan(
                        sc[j][:, :ne], mask[:, :ne], sc[j][:, :ne], 0.0,
                        op0=mybir.AluOpType.mult, op1=mybir.AluOpType.max,
                    )
                obv = ob[j].rearrange("p (r c) -> p r c", r=R, c=W)
                nc.scalar.activation(
                    obv[:, 0:n_out, ::-1],
                    sc[j].rearrange("p (r c) -> p r c", r=R, c=W)[:, 0:n_out, :],
                    mybir.ActivationFunctionType.Copy, bias=-M_OFF, scale=1.0,
                )
                nc.sync.dma_start(
                    y_d[n, :, h0 + 1 : h0 + 1 + n_out, :], obv[:, 0:n_out, :]
                )

            for n in range(IMG_PER_CORE):
                for k in range(NB):
                    h0 = H - (k + 1) * R
                    j = k % 2

                    # ---- DMA x band (rows h0-1 .. h0+R, clipped) ----
                    lo = max(h0 - 1, 0)
                    hi = min(h0 + R, H - 1)
                    slo = lo - (h0 - 1)          # first valid slot
                    nrows_in = hi - lo + 1
                    for c in range(2):
                        xv = xp_view(c, j)
                        nc.sync.dma_start(
                            xv[:, slo : slo + nrows_in, 2 : 2 + W],
                            x_d[n, c * P : (c + 1) * P, lo : hi + 1, :],
                        )
                        if k == 0:      # bottom band: slot HP-1 is h=H -> zero
                            nc.vector.tensor_copy(xv[:, HP - 1, 1 : WP - 1], zeros[:, 0 : WP - 2])
                        if k == NB - 1:  # top band: slot 0 is h=-1 -> zero
                            nc.vector.tensor_copy(xv[:, 0, 1 : WP - 1], zeros[:, 0 : WP - 2])

                    xviews = [xp_view(0, j), xp_view(1, j)]

                    # ---- conv3 for the PREVIOUS band, emitted first so its
                    # scan heads the DVE queue and the un-reverse never blocks
                    # this band's PSUM evacuations on ScalarE ----
                    if k > 0:
                        conv3_band(n, k - 1)

                    # ---- conv1 -> ub rows 0..15 (fp32, BN+ReLU) ----
                    for t in range(R // 4):
                        pu = ps.tile([P, 4 * W], F32, name="pu", tag="pu")
                        conv_mms(pu, [wu_v[:, 0], wu_v[:, 1]], xviews, 4 * t, 4)
                        nc.scalar.activation(
                            ub[j][:, 4 * t * W : 4 * (t + 1) * W], pu[:],
                            mybir.ActivationFunctionType.Relu, bias=b1, scale=s1,
                        )

                    # ---- carry row (slot 16) ----
                    if k == 0:
                        nc.vector.memset(ub[j][:, 16 * W :], 0.0)
                    else:
                        nc.vector.tensor_copy(ub[j][:, 16 * W :], ub[1 - j][:, 0:W])

                    # ---- TopPool: in-place suffix max over 17 rows ----
                    if not no_pool:
                        peng = getattr(nc, pool_engine)
                        for s in (1, 2, 4, 8, 16):
                            nrows = 17 - s
                            peng.tensor_max(
                                ub[j][:, : nrows * W],
                                ub[j][:, : nrows * W],
                                ub[j][:, s * W : 17 * W],
                            )

                    # ---- conv2 -> dn (fp32, BN+ReLU) ----
                    for t in range(R // 4):
                        pd = ps.tile([P, 4 * W], F32, name="pd", tag="pd")
                        conv_mms(pd, [wd_v[:, 0], wd_v[:, 1]], xviews, 4 * t, 4)
                        nc.scalar.activation(
                            dn[j][:, 4 * t * W : 4 * (t + 1) * W], pd[:],
                            mybir.ActivationFunctionType.Relu, bias=b2, scale=s2,
                        )

                    # ---- merge1 = pooled + down -> mg interior (f32r) ----
                    mv = mg_view(j)
                    nc.vector.tensor_add(
                        mv[:, 0:R, 2 : 2 + W], ub[j].rearrange("p (r c) -> p r c", r=17, c=W)[:, 0:R, :],
                        dn[j].rearrange("p (r c) -> p r c", r=R, c=W),
                    )
                    # halo rows 16,17 = rows 0,1 of previous band (or zeros)
                    if k == 0:
                        nc.vector.tensor_copy(mv[:, R : R + 2, 2 : 2 + W], zv[:, :, 0:W])
                    else:
                        nc.vector.tensor_copy(mv[:, R : R + 2, 2 : 2 + W], mg_view(1 - j)[:, 0:2, 2 : 2 + W])

                # last band's conv3 after the loop
                conv3_band(n, NB - 1)

                # ---- final pass: out row 0 (kh=0 reads h=-1: all-zero, skipped) ----
                mv = mg_view((NB - 1) % 2)
                p0 = ps.tile([P, 4 * W], F32, name="p0", tag="pc", bufs=2)
                for kh in (1, 2):
                    for kw in range(3):
                        nc.tensor.matmul(
                            p0[:, :W], wp_v[:, kh * 3 + kw, :],
                            mv[:, kh - 1 : kh, 1 + kw : 1 + kw + W],
                            start=(kh == 1 and kw == 0), stop=(kh == 2 and kw == 2),
                        )
                nc.scalar.activation(
                    sc0[:, ::-1], p0[:, :W],
                    mybir.ActivationFunctionType.Identity, bias=b3m, scale=s3,
                )
                nc.vector.tensor_tensor_scan(
                    sc0[:], mask[:, :W], sc0[:], 0.0,
                    op0=mybir.AluOpType.mult, op1=mybir.AluOpType.max,
                )
                nc.scalar.activation(
                    ob0[:, ::-1], sc0[:],
                    mybir.ActivationFunctionType.Copy, bias=-M_OFF, scale=1.0,
                )
                nc.sync.dma_start(y_d[n, :, 0:1, :], ob0[:].rearrange("p (r c) -> p r c", r=1, c=W))
            if rep_ctx is not None:
                rep_ctx.__exit__(None, None, None)

    _split_waits(nc, max_waits=1)
    return nc


_CACHE = {}


def _get_nc():
    if "nc" not in _CACHE:
        _CACHE["nc"] = build_nc()
    return _CACHE["nc"]


def _host_prep(w_up, up_gamma, up_beta, up_mean, up_var,
               w_down, down_gamma, down_beta, down_mean, down_var,
               w_p, p_gamma, p_beta, p_mean, p_var):
    def fold(gamma, beta, mean, var):
        inv = gamma / np.sqrt(var + EPS)
        return inv.astype(np.float32), (beta - mean * inv).astype(np.float32)

    s1, b1 = fold(up_gamma, up_beta, up_mean, up_var)
    s2, b2 = fold(down_gamma, down_beta, down_mean, down_var)
    s3, b3 = fold(p_gamma, p_beta, p_mean, p_var)
    bn = np.stack([s1, b1, s2, b2, s3, b3 + M_OFF], axis=1).astype(np.float32)

    def prep_w2(w):  # (COUT, CIN, 3, 3) -> [cin128, (chunk, s, cout128)]
        a = w.transpose(1, 2, 3, 0).reshape(2, P, 3, 3, COUT)   # (chunk,k,kh,kw,m)
        a = a.transpose(1, 0, 2, 3, 4)                          # (k,chunk,kh,kw,m)
        return np.ascontiguousarray(a.reshape(P, 2 * 9 * COUT)).astype(np.float32)

    def prep_w1(w):  # (COUT, COUT, 3, 3) -> [cin128, (s, cout128)]
        a = w.transpose(1, 2, 3, 0)                             # (k,kh,kw,m)
        return np.ascontiguousarray(a.reshape(P, 9 * COUT)).astype(np.float32)

    return prep_w2(w_up), prep_w2(w_down), prep_w1(w_p), bn


def kernel(x, w_up, up_gamma, up_beta, up_mean, up_var,
           w_down, down_gamma, down_beta, down_mean, down_var,
           w_p, p_gamma, p_beta, p_mean, p_var):
    x = np.asarray(x, dtype=np.float32)
    args = [np.asarray(a, dtype=np.float32) for a in (
        w_up, up_gamma, up_beta, up_mean, up_var,
        w_down, down_gamma, down_beta, down_mean, down_var,
        w_p, p_gamma, p_beta, p_mean, p_var)]
    wu, wd, wp, bn = _host_prep(*args)

    nc = _get_nc()
    in_maps = []
    for c in range(N_CORES):
        in_maps.append({
            "x": np.ascontiguousarray(x[c * IMG_PER_CORE : (c + 1) * IMG_PER_CORE]),
            "wu": wu, "wd": wd, "wp": wp, "bn": bn,
        })
    res = run_bass_kernel_spmd(nc, in_maps, core_ids=list(range(N_CORES)), trace=False)
    return np.concatenate([res.results[c]["y"] for c in range(N_CORES)], axis=0)


if __name__ == "__main__":
    nc = build_nc()
    n_inst = sum(len(b.instructions) for f in nc.m.functions for b in f.blocks)
    print(f"built: {n_inst} instructions")


# revision 13
# speedup vs baseline: 1.0166x; 1.0140x over previous
"""CascadeCornerPooling TRN2 kernel.

Data-parallel over batch: 16 images across 8 NeuronCores (2 per core).
Per image (NCHW, C_in=256, C_out=128, H=W=128):
    up    = relu(bn1(conv3x3(x, w_up)))
    up    = reverse-cummax over H          (TopPool)
    down  = relu(bn2(conv3x3(x, w_down)))
    merge = bn3(conv3x3(up + down, w_p))
    out   = reverse-cummax over W          (LeftPool)

Implementation: H-bands of 16 rows processed bottom-up.
 - convs: 9 shifted matmuls per Cin-chunk in float32r (full PE rate),
   accumulated in PSUM; rhs APs read padded SBUF tiles (row stride 132,
   interior at col offset 2, zeroed pad cols 1/130).
 - BN+ReLU fused into ScalarE PSUM evacuation (per-channel scale/bias APs).
 - TopPool: in-place log-shift suffix-max over 17 rows (16 band rows +
   carry row holding the pooled first row of the band below).
 - LeftPool: BN3 evacuation writes each row W-reversed with +M offset;
   one masked tensor_tensor_scan (state = max(mask*state, data)) per band
   does the segmented reverse cummax; ScalarE un-reverses and subtracts M.
"""

import numpy as np

import concourse.bass as bass
import concourse.tile as tile
from concourse import mybir
from concourse.bass_utils import run_bass_kernel_spmd

F32 = mybir.dt.float32
F32R = mybir.dt.float32r

N_CORES = 8
IMG_PER_CORE = 2
CIN, COUT = 256, 128
H = W = 128
P = 128          # partitions
R = 16           # band rows
NB = H // R      # bands per image
WP = W + 4       # padded row stride (input col w at offset 2+w; cols 1,130 zero)
HP = R + 2       # band rows + 2 halo rows
M_OFF = 128.0    # positivity offset for the LeftPool scan
EPS = 1e-5


def _split_waits(nc, max_waits=1):
    """This container's walrus rejects >1 sync-wait per instruction; hoist
    excess waits onto same-engine NOPs inserted just before."""
    for f in nc.m.functions:
        for b in f.blocks:
            new_insts = []
            for inst in b.instructions:
                si = inst.sync_info
                if si is not None and si.on_wait and len(si.on_wait) > max_waits:
                    waits = list(si.on_wait)
                    head, tail_w = waits[:-max_waits], waits[-max_waits:]
                    for ci in range(0, len(head), max_waits):
                        new_insts.append(
                            mybir.InstNoOp(
                                name=f"{inst.name}-wsplit{ci}",
                                engine=inst.engine,
                                bass_nofuse=True,
                                sync_info=mybir.SyncInfo(
                                    on_wait=head[ci : ci + max_waits], on_update=[]
                                ),
                            )
                        )
                    inst.sync_info = mybir.SyncInfo(
                        on_wait=tail_w, on_update=list(si.on_update)
                    )
                new_insts.append(inst)
            b.instructions[:] = new_insts


def build_nc(nrep=1, no_pool=False, no_scan=False, pool_engine="vector"):
    nc = bass.Bass("TRN2", target_bir_lowering=False, debug=False)

    x_d = nc.dram_tensor("x", [IMG_PER_CORE, CIN, H, W], F32R, kind="ExternalInput").ap()
    wu_d = nc.dram_tensor("wu", [P, 2 * 9 * P], F32R, kind="ExternalInput").ap()
    wd_d = nc.dram_tensor("wd", [P, 2 * 9 * P], F32R, kind="ExternalInput").ap()
    wp_d = nc.dram_tensor("wp", [P, 9 * P], F32R, kind="ExternalInput").ap()
    bn_d = nc.dram_tensor("bn", [P, 6], F32, kind="ExternalInput").ap()  # s1 b1 s2 b2 s3 b3m
    y_d = nc.dram_tensor("y", [IMG_PER_CORE, COUT, H, W], F32, kind="ExternalOutput").ap()

    with tile.TileContext(nc) as tc:
        with (
            tc.tile_pool(name="const", bufs=1) as cp,
            tc.tile_pool(name="band", bufs=1) as bp,
            tc.tile_pool(name="ps", bufs=3, space="PSUM") as ps,
        ):
            # ---- constants (wu first: first conv needs it immediately) ----
            wu_t = cp.tile([P, 2 * 9 * P], F32R)
            nc.sync.dma_start(wu_t[:, : 9 * P], wu_d[:, : 9 * P])
            nc.sync.dma_start(wu_t[:, 9 * P :], wu_d[:, 9 * P :])
            bn_early = True
            wd_t = cp.tile([P, 2 * 9 * P], F32R)
            wp_t = cp.tile([P, 9 * P], F32R)
            wu_v = wu_t.rearrange("k (c s m) -> k c s m", c=2, s=9, m=P)
            wd_v = wd_t.rearrange("k (c s m) -> k c s m", c=2, s=9, m=P)
            wp_v = wp_t.rearrange("k (s m) -> k s m", s=9, m=P)

            bn_t = cp.tile([P, 6], F32)
            nc.sync.dma_start(bn_t[:], bn_d[:])
            s1, b1 = bn_t[:, 0:1], bn_t[:, 1:2]
            s2, b2 = bn_t[:, 2:3], bn_t[:, 3:4]
            s3, b3m = bn_t[:, 4:5], bn_t[:, 5:6]

            zeros = cp.tile([P, 2 * WP], F32)
            nc.vector.memset(zeros[:], 0.0)
            zv = zeros.rearrange("p (r c) -> p r c", r=2, c=WP)

            mask = cp.tile([P, R * W], F32)
            nc.vector.memset(mask[:], 1.0)
            nc.vector.memset(mask[:, 0::W], 0.0)

            # ---- band tiles (manual ping-pong) ----
            xp = [[bp.tile([P, HP * WP], F32R, name=f"xp{c}{j}", tag=f"xp{c}{j}") for j in range(2)] for c in range(2)]
            ub = [bp.tile([P, 17 * W], F32, name=f"ub{j}", tag=f"ub{j}") for j in range(2)]
            dn = [bp.tile([P, R * W], F32, name=f"dn{j}", tag=f"dn{j}") for j in range(2)]
            mg = [bp.tile([P, HP * WP], F32R, name=f"mg{j}", tag=f"mg{j}") for j in range(2)]
            sc = [bp.tile([P, R * W], F32, name=f"sc{j}", tag=f"sc{j}") for j in range(2)]
            ob = [bp.tile([P, R * W], F32, name=f"ob{j}", tag=f"ob{j}") for j in range(2)]
            sc0 = bp.tile([P, W], F32)
            ob0 = bp.tile([P, W], F32)

            # zero the pad columns (1 and WP-2) of padded tiles once;
            # DMAs/adds only ever write interiors afterwards.
            for t_ in [xp[0][0], xp[0][1], xp[1][0], xp[1][1], mg[0], mg[1]]:
                v = t_.rearrange("p (r c) -> p r c", r=HP, c=WP)
                nc.vector.tensor_copy(
                    v[:, :, 1 : WP - 1 : WP - 3],
                    zeros.rearrange("p (r c) -> p r c", r=WP, c=2)[:, 0:HP, :],
                )

            def xp_view(c, j):
                return xp[c][j].rearrange("p (r c) -> p r c", r=HP, c=WP)

            def mg_view(j):
                return mg[j].rearrange("p (r c) -> p r c", r=HP, c=WP)

            rep_ctx = tc.For_i(0, nrep, 1) if nrep > 1 else None

            def conv_mms(psum, w_chunks, src_views, t0, nrows, first_extra=None):
                """Accumulate 9*len(chunks) matmuls into psum.
                src row slot for out-row r (band-relative) and kh is r+kh."""
                n_ch = len(w_chunks)
                for ci in range(n_ch):
                    for kh in range(3):
                        for kw in range(3):
                            s = kh * 3 + kw
                            rhs = src_views[ci][:, t0 + kh : t0 + kh + nrows, 1 + kw : 1 + kw + W]
                            nc.tensor.matmul(
                                psum[:, : nrows * W],
                                w_chunks[ci][:, s, :],
                                rhs,
                                start=(ci == 0 and s == 0),
                                stop=(ci == n_ch - 1 and s == 8),
                            )

            if rep_ctx is not None:
                rep_ctx.__enter__()

            def conv3_band(n, k):
                """conv3 + LeftPool + output DMA for band k (lagged one band)."""
                h0 = H - (k + 1) * R
                j = k % 2
                mv = mg_view(j)
                n_out = R if k > 0 else R - 1
                q = 0
                while q < n_out:
                    nr = min(4, n_out - q)
                    pc = ps.tile([P, 4 * W], F32, name="pc", tag="pc", bufs=2)
                    conv_mms(pc, [wp_v], [mv], q, nr)
                    scv = sc[j].rearrange("p (r c) -> p r c", r=R, c=W)
                    nc.scalar.activation(
                        scv[:, q : q + nr, ::-1], pc[:, : nr * W],
                        mybir.ActivationFunctionType.Identity, bias=b3m, scale=s3,
                    )
                    q += nr
                ne = n_out * W
                if not no_scan:
                    nc.vector.tensor_tensor_scan(
                        sc[j][:, :ne], mask[:, :ne], sc[j][:, :ne], 0.0,
                        op0=mybir.AluOpType.mult, op1=mybir.AluOpType.max,
                    )
                obv = ob[j].rearrange("p (r c) -> p r c", r=R, c=W)
                nc.scalar.activation(
                    obv[:, 0:n_out, ::-1],
                    sc[j].rearrange("p (r c) -> p r c", r=R, c=W)[:, 0:n_out, :],
                    mybir.ActivationFunctionType.Copy, bias=-M_OFF, scale=1.0,
                )
                nc.sync.dma_start(
                    y_d[n, :, h0 + 1 : h0 + 1 + n_out, :], obv[:, 0:n_out, :]
                )

            for n in range(IMG_PER_CORE):
                for k in range(NB):
                    h0 = H - (k + 1) * R
                    j = k % 2

                    # ---- DMA x band (rows h0-1 .. h0+R, clipped) ----
                    lo = max(h0 - 1, 0)
                    hi = min(h0 + R, H - 1)
                    slo = lo - (h0 - 1)          # first valid slot
                    nrows_in = hi - lo + 1
                    for c in range(2):
                        xv = xp_view(c, j)
                        if n == 0 and k == 0:
                            # split the very first transfers so the first conv
                            # groups can start before the whole band lands
                            half = 7
                            nc.sync.dma_start(
                                xv[:, slo : slo + half, 2 : 2 + W],
                                x_d[n, c * P : (c + 1) * P, lo : lo + half, :],
                            )
                            nc.sync.dma_start(
                                xv[:, slo + half : slo + nrows_in, 2 : 2 + W],
                                x_d[n, c * P : (c + 1) * P, lo + half : hi + 1, :],
                            )
                        else:
                            nc.sync.dma_start(
                                xv[:, slo : slo + nrows_in, 2 : 2 + W],
                                x_d[n, c * P : (c + 1) * P, lo : hi + 1, :],
                            )
                        if k == 0:      # bottom band: slot HP-1 is h=H -> zero
                            nc.vector.tensor_copy(xv[:, HP - 1, 1 : WP - 1], zeros[:, 0 : WP - 2])
                        if k == NB - 1:  # top band: slot 0 is h=-1 -> zero
                            nc.vector.tensor_copy(xv[:, 0, 1 : WP - 1], zeros[:, 0 : WP - 2])

                    xviews = [xp_view(0, j), xp_view(1, j)]

                    # ---- conv3 for the PREVIOUS band, emitted first so its
                    # scan heads the DVE queue and the un-reverse never blocks
                    # this band's PSUM evacuations on ScalarE ----
                    if k > 0:
                        conv3_band(n, k - 1)

                    # ---- conv1 -> ub rows 0..15 (fp32, BN+ReLU) ----
                    for t in range(R // 4):
                        pu = ps.tile([P, 4 * W], F32, name="pu", tag="pu")
                        conv_mms(pu, [wu_v[:, 0], wu_v[:, 1]], xviews, 4 * t, 4)
                        nc.scalar.activation(
                            ub[j][:, 4 * t * W : 4 * (t + 1) * W], pu[:],
                            mybir.ActivationFunctionType.Relu, bias=b1, scale=s1,
                        )

                    if n == 0 and k == 0:
                        # deferred weight DMAs: needed from conv2/conv3 of band
                        # 0 onward; emitting here keeps them out of the first
                        # x-band transfer's path
                        nc.sync.dma_start(wd_t[:], wd_d[:])
                        nc.sync.dma_start(wp_t[:], wp_d[:])

                    # ---- carry row (slot 16) ----
                    if k == 0:
                        nc.vector.memset(ub[j][:, 16 * W :], 0.0)
                    else:
                        nc.vector.tensor_copy(ub[j][:, 16 * W :], ub[1 - j][:, 0:W])

                    # ---- TopPool: in-place suffix max over 17 rows ----
                    if not no_pool:
                        peng = getattr(nc, pool_engine)
                        for s in (1, 2, 4, 8, 16):
                            nrows = 17 - s
                            peng.tensor_max(
                                ub[j][:, : nrows * W],
                                ub[j][:, : nrows * W],
                                ub[j][:, s * W : 17 * W],
                            )

                    # ---- conv2 -> dn (fp32, BN+ReLU) ----
                    for t in range(R // 4):
                        pd = ps.tile([P, 4 * W], F32, name="pd", tag="pd")
                        conv_mms(pd, [wd_v[:, 0], wd_v[:, 1]], xviews, 4 * t, 4)
                        nc.scalar.activation(
                            dn[j][:, 4 * t * W : 4 * (t + 1) * W], pd[:],
                            mybir.ActivationFunctionType.Relu, bias=b2, scale=s2,
                        )

                    # ---- merge1 = pooled + down -> mg interior (f32r) ----
                    mv = mg_view(j)
                    nc.vector.tensor_add(
                        mv[:, 0:R, 2 : 2 + W], ub[j].rearrange("p (r c) -> p r c", r=17, c=W)[:, 0:R, :],
                        dn[j].rearrange("p (r c) -> p r c", r=R, c=W),
                    )
                    # halo rows 16,17 = rows 0,1 of previous band (or zeros)
                    if k == 0:
                        nc.vector.tensor_copy(mv[:, R : R + 2, 2 : 2 + W], zv[:, :, 0:W])
                    else:
                        nc.vector.tensor_copy(mv[:, R : R + 2, 2 : 2 + W], mg_view(1 - j)[:, 0:2, 2 : 2 + W])

                # last band's conv3 after the loop
                conv3_band(n, NB - 1)

                # ---- final pass: out row 0 (kh=0 reads h=-1: all-zero, skipped) ----
                mv = mg_view((NB - 1) % 2)
                p0 = ps.tile([P, 4 * W], F32, name="p0", tag="pc", bufs=2)
                for kh in (1, 2):
                    for kw in range(3):
                        nc.tensor.matmul(
                            p0[:, :W], wp_v[:, kh * 3 + kw, :],
                            mv[:, kh - 1 : kh, 1 + kw : 1 + kw + W],
                            start=(kh == 1 and kw == 0), stop=(kh == 2 and kw == 2),
                        )
                nc.scalar.activation(
                    sc0[:, ::-1], p0[:, :W],
                    mybir.ActivationFunctionType.Identity, bias=b3m, scale=s3,
                )
                nc.vector.tensor_tensor_scan(
                    sc0[:], mask[:, :W], sc0[:], 0.0,
                    op0=mybir.AluOpType.mult, op1=mybir.AluOpType.max,
                )
                nc.scalar.activation(
                    ob0[:, ::-1], sc0[:],
                    mybir.ActivationFunctionType.Copy, bias=-M_OFF, scale=1.0,
                )
                nc.sync.dma_start(y_d[n, :, 0:1, :], ob0[:].rearrange("p (r c) -> p r c", r=1, c=W))
            if rep_ctx is not None:
                rep_ctx.__exit__(None, None, None)

    _split_waits(nc, max_waits=1)
    return nc


_CACHE = {}


def _get_nc():
    if "nc" not in _CACHE:
        _CACHE["nc"] = build_nc()
    return _CACHE["nc"]


def _host_prep(w_up, up_gamma, up_beta, up_mean, up_var,
               w_down, down_gamma, down_beta, down_mean, down_var,
               w_p, p_gamma, p_beta, p_mean, p_var):
    def fold(gamma, beta, mean, var):
        inv = gamma / np.sqrt(var + EPS)
        return inv.astype(np.float32), (beta - mean * inv).astype(np.float32)

    s1, b1 = fold(up_gamma, up_beta, up_mean, up_var)
    s2, b2 = fold(down_gamma, down_beta, down_mean, down_var)
    s3, b3 = fold(p_gamma, p_beta, p_mean, p_var)
    bn = np.stack([s1, b1, s2, b2, s3, b3 + M_OFF], axis=1).astype(np.float32)

    def prep_w2(w):  # (COUT, CIN, 3, 3) -> [cin128, (chunk, s, cout128)]
        a = w.transpose(1, 2, 3, 0).reshape(2, P, 3, 3, COUT)   # (chunk,k,kh,kw,m)
        a = a.transpose(1, 0, 2, 3, 4)                          # (k,chunk,kh,kw,m)
        return np.ascontiguousarray(a.reshape(P, 2 * 9 * COUT)).astype(np.float32)

    def prep_w1(w):  # (COUT, COUT, 3, 3) -> [cin128, (s, cout128)]
        a = w.transpose(1, 2, 3, 0)                             # (k,kh,kw,m)
        return np.ascontiguousarray(a.reshape(P, 9 * COUT)).astype(np.float32)

    return prep_w2(w_up), prep_w2(w_down), prep_w1(w_p), bn


def kernel(x, w_up, up_gamma, up_beta, up_mean, up_var,
           w_down, down_gamma, down_beta, down_mean, down_var,
           w_p, p_gamma, p_beta, p_mean, p_var):
    x = np.asarray(x, dtype=np.float32)
    args = [np.asarray(a, dtype=np.float32) for a in (
        w_up, up_gamma, up_beta, up_mean, up_var,
        w_down, down_gamma, down_beta, down_mean, down_var,
        w_p, p_gamma, p_beta, p_mean, p_var)]
    wu, wd, wp, bn = _host_prep(*args)

    nc = _get_nc()
    in_maps = []
    for c in range(N_CORES):
        in_maps.append({
            "x": np.ascontiguousarray(x[c * IMG_PER_CORE : (c + 1) * IMG_PER_CORE]),
            "wu": wu, "wd": wd, "wp": wp, "bn": bn,
        })
    res = run_bass_kernel_spmd(nc, in_maps, core_ids=list(range(N_CORES)), trace=False)
    return np.concatenate([res.results[c]["y"] for c in range(N_CORES)], axis=0)


if __name__ == "__main__":
    nc = build_nc()
    n_inst = sum(len(b.instructions) for f in nc.m.functions for b in f.blocks)
    print(f"built: {n_inst} instructions")
